# revision 10
# baseline (speedup 1.0000x reference)
"""ARMA GNN (2-layer, K=2 stacks) Trainium2 kernel.

Strategy (8-core SPMD, node-sharded), v4 "aggregate-then-transform":
  * norm folding: norm[e] = dinv[row]*dinv[col], and aggregation is linear,
    so each layer aggregates RAW scaled features and applies the weight
    matrix per 128-node window afterwards:
        xagg[n] = sum_{e: col=n} (dinv[row]*feat[row])
        out_k   = relu(0.5*(dinv[n]*(xagg @ Wk) + feat@RWk + bk)); mean_k
  * Layer-1 gather table is the host-prepared dinv*x rows (fp16, 128-wide,
    gathered straight from the input tensor).  Layer-2 table is
    dinv*h1 rows (f32, 64-wide) built from the AllGathered node-major h1
    with a single scale per chunk - no matmuls between the layers.
  * Edges sharded by target; per 128-edge block: dma_gather (SWDGE, 4
    queues round-robin across Q7 core pairs), selection matrix
    S[e,slot]=(iota==slot[e]) built on DVE (is_equal) and ACT
    (relu(1-|iota-slot|)) in parallel, matmul S.T @ M accumulated in PSUM.
  * Per window epilogue: evict xagg, PE-transpose, matmul with Wcat,
    relu-epilogue, pack node-major output.
  * One AllGather of node-major fp16 h1 between layers; root2 is built
    (via PE transposes of h1) while the collective runs.

kernel(**inputs) takes the FULL problem inputs and returns the FULL output.
"""

import sys

sys.path.insert(0, "/opt/trn_rl_repo")

from contextlib import ExitStack

import numpy as np

P = 128


class Cfg:
    def __init__(self, N, NC, SHARD, B0, ZPAD=256, WB=2, G=8, SUPER=4,
                 FIN=128, HID=64, FOUT=64, K=2, ACT_FRAC=0.4):
        self.N, self.NC, self.SHARD, self.B0, self.ZPAD = N, NC, SHARD, B0, ZPAD
        self.WB, self.G, self.SUPER = WB, G, SUPER
        self.FIN, self.HID, self.FOUT, self.K = FIN, HID, FOUT, K
        self.ACT_FRAC = ACT_FRAC
        self.NSTAR = NC * SHARD
        self.W = SHARD // P
        self.ROWS = self.NSTAR + ZPAD
        self.HALFA = B0 + ZPAD
        self.HALFB = self.NSTAR - B0
        self.NCHUNK = self.ROWS // P
        self.ACH = self.HALFA // P
        assert B0 % P == 0 and SHARD % P == 0 and ZPAD % P == 0
        assert self.HALFA < 32768 and self.HALFB < 32768
        assert N > B0 and N < self.NSTAR
        assert K * HID == 128 and K * FOUT == 128 and FIN == 128
        self.ZLOCA = B0
        self.ZLOCB = (N + ZPAD) - self.HALFA
        self.batches = [tuple(range(b, min(b + WB, self.W)))
                        for b in range(0, self.W, WB)]
        self.sbatches = [self.batches[i:i + SUPER]
                         for i in range(0, len(self.batches), SUPER)]


REAL = dict(N=50000, NC=8, SHARD=6272, B0=24960)


# --------------------------------------------------------------------------
# host preprocessing
# --------------------------------------------------------------------------
def _preprocess(c: Cfg, x, edge_index, init_w1, root_w1, b1, init_w2, root_w2, b2):
    N, NC, SHARD = c.N, c.NC, c.SHARD
    row = np.asarray(edge_index[0]).astype(np.int64)
    col = np.asarray(edge_index[1]).astype(np.int64)
    x = np.asarray(x, dtype=np.float32)

    deg = np.bincount(col, minlength=N).astype(np.float64)
    dinv = np.where(deg > 0, deg ** -0.5, 0.0).astype(np.float32)
    dinv_full = np.zeros(c.NSTAR, np.float32)
    dinv_full[:N] = dinv

    srow = row + (row >= c.B0) * c.ZPAD

    percore = []
    counts = np.zeros((NC, c.W, 2), np.int64)
    for cc in range(NC):
        base = cc * SHARD
        m = (col >= base) & (col < base + SHARD)
        ec = (col[m] - base).astype(np.int64)
        es = srow[m]
        half = (es >= c.HALFA).astype(np.int64)
        key = (ec >> 7) * 2 + half
        order = np.argsort(key, kind="stable")
        ec, es, key = ec[order], es[order], key[order]
        bounds = np.searchsorted(key, np.arange(2 * c.W + 1))
        percore.append((ec, es, bounds))
        for w in range(c.W):
            counts[cc, w, 0] = bounds[2 * w + 1] - bounds[2 * w]
            counts[cc, w, 1] = bounds[2 * w + 2] - bounds[2 * w + 1]

    NBA = [max(1, int(-(-counts[:, w, 0].max() // P))) for w in range(c.W)]
    NBB = [max(1, int(-(-counts[:, w, 1].max() // P))) for w in range(c.W)]

    def build_stream(cc, half_id, NB_list):
        ec, es, bounds = percore[cc]
        toks, slots = [], []
        zloc = c.ZLOCA if half_id == 0 else c.ZLOCB
        for batch in c.batches:
            for w in batch:
                lo, hi = bounds[2 * w + half_id], bounds[2 * w + half_id + 1]
                k = hi - lo
                n = NB_list[w] * P
                t = np.full(n, zloc, np.int64)
                s = np.zeros(n, np.int64)
                t[:k] = es[lo:hi] - (c.HALFA if half_id else 0)
                s[:k] = ec[lo:hi] & 127
                toks.append(t)
                slots.append(s)
        toks = np.concatenate(toks)
        slots = np.concatenate(slots).astype(np.float32)
        L = len(toks)
        idxw = np.tile(toks.reshape(L // 16, 16).T.astype(np.int16), (8, 1))
        colf = slots.reshape(L // P, P).T  # [128, NB] f32
        return np.ascontiguousarray(idxw), np.ascontiguousarray(colf)

    # layer-1 gather table: dinv*x in table-row order, fp16
    xs = np.zeros((c.ROWS, c.FIN), np.float32)
    xs[:c.B0] = x[:c.B0] * dinv[:c.B0, None]
    xs[c.HALFA:c.HALFA + (N - c.B0)] = x[c.B0:N] * dinv[c.B0:N, None]
    xs16 = np.ascontiguousarray(xs.astype(np.float16))

    dinvrow = np.zeros(c.ROWS, np.float32)
    dinvrow[:c.B0] = dinv_full[:c.B0]
    dinvrow[c.HALFA:] = dinv_full[c.B0:]
    dinvr = np.ascontiguousarray(dinvrow.reshape(c.NCHUNK, P).T)

    xpad = np.zeros((c.NSTAR, c.FIN), np.float32)
    xpad[:N] = x

    def cat2(w, dt):
        w = np.asarray(w, dtype=np.float32)
        return np.ascontiguousarray(np.concatenate([w[0], w[1]], axis=1).astype(dt))

    w1cat = cat2(init_w1, np.float32)            # [128,128] f32
    w2cat = cat2(init_w2, np.float32)            # [64,128]  f32
    rw1c = cat2(0.5 * np.asarray(root_w1, np.float32), np.float16)
    rw2c = cat2(0.5 * np.asarray(root_w2, np.float32), np.float32)
    b1 = np.asarray(b1, dtype=np.float32)
    b2 = np.asarray(b2, dtype=np.float32)
    b1b = np.ascontiguousarray(np.tile(0.5 * np.concatenate([b1[0], b1[1]]), (P, 1)))
    b2b = np.ascontiguousarray(np.tile(0.5 * np.concatenate([b2[0], b2[1]]), (P, 1)))

    in_maps = []
    for cc in range(NC):
        base = cc * SHARD
        idxA, colfA32 = build_stream(cc, 0, NBA)
        idxB, colfB32 = build_stream(cc, 1, NBB)
        dinvo = 0.5 * dinv_full[base:base + SHARD].reshape(c.W, P).T
        in_maps.append({
            "xs": xs16,
            "xTow": np.ascontiguousarray(xpad[base:base + SHARD].T.astype(np.float16)),
            "w1cat": w1cat, "rw1c": rw1c, "w2cat": w2cat, "rw2c": rw2c,
            "b1b": b1b, "b2b": b2b,
            "dinvr": dinvr,
            "dinvo": np.ascontiguousarray(dinvo.astype(np.float32)),
            "idxA": idxA, "idxB": idxB,
            "colfA32": colfA32, "colfB32": colfB32,
        })
    return in_maps, NBA, NBB


# --------------------------------------------------------------------------
# device program
# --------------------------------------------------------------------------
def _build_program(c: Cfg, NBA, NBB):
    import concourse.tile as tile
    from concourse import bacc, mybir
    from concourse.masks import make_identity

    f32 = mybir.dt.float32
    f16 = mybir.dt.float16
    i16 = mybir.dt.int16
    AL = mybir.AluOpType
    AF = mybir.ActivationFunctionType

    NBAtot, NBBtot = sum(NBA), sum(NBB)
    LA, LB = NBAtot * P, NBBtot * P

    nc = bacc.Bacc("TRN2", target_bir_lowering=False, debug=False,
                   num_devices=c.NC, num_swdge_queues=4)
    qrr = [0]

    def din(name, shape, dt=f32):
        return nc.dram_tensor(name, shape, dt, kind="ExternalInput")

    xs = din("xs", [c.ROWS, 128], f16)           # layer-1 gather table
    xTow = din("xTow", [P, c.SHARD], f16)
    w1cat = din("w1cat", [P, 128], f32)
    rw1c = din("rw1c", [P, 128], f16)
    w2cat = din("w2cat", [64, 128], f32)
    rw2c = din("rw2c", [64, 128], f32)
    b1b = din("b1b", [P, 128]); b2b = din("b2b", [P, 128])
    dinvr = din("dinvr", [P, c.NCHUNK])
    dinvo = din("dinvo", [P, c.W])
    idxA = din("idxA", [P, LA // 16], i16)
    idxB = din("idxB", [P, LB // 16], i16)
    colfA32 = din("colfA32", [P, NBAtot], f32)
    colfB32 = din("colfB32", [P, NBBtot], f32)
    yt = nc.dram_tensor("yt", [c.SHARD, 64], f32, kind="ExternalOutput")

    g1tA = nc.dram_tensor("g1tA", [c.HALFA, 128], f16)  # layer-2 gather table
    g1tB = nc.dram_tensor("g1tB", [c.HALFB, 128], f16)
    ccin = nc.dram_tensor("ccin", [c.SHARD, 64], f16)
    ccout = nc.dram_tensor("ccout", [c.NC, c.SHARD, 64], f16)

    # table2 chunk -> (shard, local chunk); zero chunks -> None
    chunk_src = [None] * c.NCHUNK
    for s in range(c.NC):
        for j in range(c.W):
            node0 = s * c.SHARD + j * P
            r0 = node0 + (c.ZPAD if node0 >= c.B0 else 0)
            chunk_src[r0 // P] = (s, j)
    runs = []
    i = 0
    while i < c.NCHUNK:
        if chunk_src[i] is None:
            j = i
            while j < c.NCHUNK and chunk_src[j] is None and (j - i) < 12:
                j += 1
            runs.append(("zero", i, j - i, 0))
            i = j
        else:
            s0, l0 = chunk_src[i]
            j = i
            while (j < c.NCHUNK and chunk_src[j] is not None
                   and chunk_src[j][0] == s0
                   and chunk_src[j][1] == l0 + (j - i) and (j - i) < 12):
                j += 1
            runs.append((s0, i, j - i, l0))
            i = j

    with tile.TileContext(nc) as tc, ExitStack() as ctx:
        cpool = ctx.enter_context(tc.tile_pool(name="consts", bufs=1))
        xtp = ctx.enter_context(tc.tile_pool(name="xtp", bufs=3))
        stg = ctx.enter_context(tc.tile_pool(name="stg", bufs=3))
        gth = ctx.enter_context(tc.tile_pool(name="gth", bufs=5))
        sgp = ctx.enter_context(tc.tile_pool(name="sgp", bufs=8))
        idxp = ctx.enter_context(tc.tile_pool(name="idxp", bufs=3))
        epi = ctx.enter_context(tc.tile_pool(name="epi", bufs=3))
        big = ctx.enter_context(tc.tile_pool(name="big", bufs=1))
        shp = ctx.enter_context(tc.tile_pool(name="shp", bufs=3))
        psx = ctx.enter_context(tc.tile_pool(name="psx", bufs=2, space="PSUM"))
        psw = ctx.enter_context(tc.tile_pool(name="psw", bufs=3, space="PSUM"))
        psy = ctx.enter_context(tc.tile_pool(name="psy", bufs=2, space="PSUM"))

        ident = cpool.tile([P, P], f32, tag="ident")
        make_identity(nc, ident[:])
        iota_i = cpool.tile([P, c.G * P], mybir.dt.int32, tag="iotai")
        nc.gpsimd.iota(iota_i[:], pattern=[[0, c.G], [1, P]], base=0,
                       channel_multiplier=0)
        iota_32 = cpool.tile([P, c.G * P], f32, tag="iota32")
        nc.vector.tensor_copy(iota_32[:], iota_i[:])

        def load_const(dram, shape, tag, dt=f32):
            t = cpool.tile(shape, dt, tag=tag)
            nc.sync.dma_start(t[:], dram[:, :])
            return t

        w1_s = load_const(w1cat, [P, 128], "w1")
        rw1_s = load_const(rw1c, [P, 128], "rw1", f16)
        w2_s = load_const(w2cat, [64, 128], "w2")
        rw2_s = load_const(rw2c, [64, 128], "rw2")
        b1_s = load_const(b1b, [P, 128], "b1")
        b2_s = load_const(b2b, [P, 128], "b2")
        dinvr_s = load_const(dinvr, [P, c.NCHUNK], "dinvr")
        dinvo_s = load_const(dinvo, [P, c.W], "dinvo")
        cA32 = load_const(colfA32, [P, NBAtot], "cA32")
        cB32 = load_const(colfB32, [P, NBBtot], "cB32")

        # ---- prolog: root1 only ----
        with nc.named_scope("prolog"):
            root1 = big.tile([P, c.SHARD], f32, tag="root")
            i = 0
            while i < c.W:
                n = min(8, c.W - i)
                xp = xtp.tile([P, 8 * 128], f16, tag="xtp")
                nc.sync.dma_start(xp[:, :n * 128], xTow[:, i * P:(i + n) * P])
                for j in range(n):
                    ps = psx.tile([P, 128], f32, tag="px")
                    nc.tensor.matmul(out=ps[:], lhsT=xp[:, j * 128:(j + 1) * 128],
                                     rhs=rw1_s[:], start=True, stop=True)
                    nc.vector.tensor_tensor(
                        out=root1[:, (i + j) * 128:(i + j + 1) * 128],
                        in0=ps[:], in1=b1_s[:], op=AL.add)
                i += n

        # ---- generic layer ----
        def layer(tabA_ap, tabB_ap, root_t, out_t, elem, xw, m_dt, wcat_s):
            s_dt = m_dt
            blkA = blkB = 0
            tokA = tokB = 0

            def build_s(n_blk, blk0, colf_s):
                tiles = []
                for g0 in range(0, n_blk, c.G):
                    gl = min(c.G, n_blk - g0)
                    s_t = sgp.tile([P, c.G * 128], s_dt, tag="sg")
                    nc.vector.tensor_tensor(
                        out=s_t[:, :gl * 128], in0=iota_32[:, :gl * 128],
                        in1=colf_s[:, blk0 + g0:blk0 + g0 + gl]
                            .to_broadcast([P, gl, 128]),
                        op=AL.is_equal)
                    tiles.append(s_t)
                return tiles

            for sb in c.sbatches:
                sbA = sum(NBA[w] for b in sb for w in b) * P
                sbB = sum(NBB[w] for b in sb for w in b) * P
                ixA = idxp.tile([P, sbA // 16], i16, tag="ixA")
                nc.sync.dma_start(ixA[:], idxA[:, tokA // 16:(tokA + sbA) // 16])
                ixB = idxp.tile([P, sbB // 16], i16, tag="ixB")
                nc.sync.dma_start(ixB[:], idxB[:, tokB // 16:(tokB + sbB) // 16])
                lA = lB = 0
                for batch in sb:
                    nA = sum(NBA[w] for w in batch)
                    nB = sum(NBB[w] for w in batch)
                    gA = gth.tile([P, nA * elem], m_dt, tag="gath")
                    nc.gpsimd.dma_gather(
                        out_ap=gA[:].rearrange("p (b f) -> p b f", f=elem),
                        in_ap=tabA_ap,
                        idxs_ap=ixA[:, lA // 16:(lA + nA * P) // 16],
                        num_idxs=nA * P, num_idxs_reg=nA * P, elem_size=elem,
                        single_packet=False, queue_num=qrr[0] % 4)
                    qrr[0] += 1
                    gB = gth.tile([P, nB * elem], m_dt, tag="gath")
                    nc.gpsimd.dma_gather(
                        out_ap=gB[:].rearrange("p (b f) -> p b f", f=elem),
                        in_ap=tabB_ap,
                        idxs_ap=ixB[:, lB // 16:(lB + nB * P) // 16],
                        num_idxs=nB * P, num_idxs_reg=nB * P, elem_size=elem,
                        single_packet=False, queue_num=qrr[0] % 4)
                    qrr[0] += 1
                    sA = build_s(nA, blkA, cA32)
                    sB_ = build_s(nB, blkB, cB32)
                    oA = oB = 0
                    for w in batch:
                        pw = psw.tile([P, elem], f32)
                        nmm = NBA[w] + NBB[w]
                        k = 0
                        for j in range(NBA[w]):
                            b = oA + j
                            nc.tensor.matmul(
                                out=pw[:],
                                lhsT=sA[b // c.G][:, (b % c.G) * 128:(b % c.G + 1) * 128],
                                rhs=gA[:, b * elem:(b + 1) * elem],
                                start=(k == 0), stop=(k == nmm - 1))
                            k += 1
                        for j in range(NBB[w]):
                            b = oB + j
                            nc.tensor.matmul(
                                out=pw[:],
                                lhsT=sB_[b // c.G][:, (b % c.G) * 128:(b % c.G + 1) * 128],
                                rhs=gB[:, b * elem:(b + 1) * elem],
                                start=(k == 0), stop=(k == nmm - 1))
                            k += 1
                        oA += NBA[w]; oB += NBB[w]
                        # window transform: xagg @ Wcat
                        u = epi.tile([P, xw], f32, tag="u")
                        nc.scalar.copy(u[:], pw[:, :xw])
                        up = psx.tile([P, 128], f32, tag="px")
                        nc.tensor.transpose(out=up[:xw, :], in_=u[:],
                                            identity=ident[:])
                        utc = epi.tile([P, 128], f32, tag="utc")
                        nc.scalar.copy(utc[:xw, :], up[:xw, :])
                        pw2 = psy.tile([P, 128], f32, tag="pw2")
                        nc.tensor.matmul(out=pw2[:], lhsT=utc[:xw, :],
                                         rhs=wcat_s[:], start=True, stop=True)
                        t2 = epi.tile([P, 128], f32, tag="t2")
                        nc.vector.scalar_tensor_tensor(
                            out=t2[:], in0=pw2[:], scalar=dinvo_s[:, w:w + 1],
                            in1=root_t[:, w * 128:(w + 1) * 128],
                            op0=AL.mult, op1=AL.add)
                        t3 = epi.tile([P, 128], f32, tag="t3")
                        nc.scalar.activation(t3[:], t2[:], AF.Relu)
                        nc.vector.tensor_tensor(
                            out=out_t[:, w * 64:(w + 1) * 64],
                            in0=t3[:, :64], in1=t3[:, 64:], op=AL.add)
                    blkA += nA; blkB += nB
                    lA += nA * P; lB += nB * P
                tokA += sbA; tokB += sbB

        h1n = big.tile([P, c.W * 64], f16, tag="ht")
        with nc.named_scope("layer1"):
            layer(xs[0:c.HALFA, :], xs[c.HALFA:c.ROWS, :], root1, h1n, 128,
                  128, f16, w1_s)

        with nc.named_scope("cc"):
            nc.sync.dma_start(
                ccin[:, :].rearrange("(w p) f -> p w f", p=P), h1n[:])
            nc.gpsimd.collective_compute(
                "AllGather", AL.bypass,
                replica_groups=[list(range(c.NC))],
                ins=[ccin.ap().opt()], outs=[ccout.ap().opt()])

        with nc.named_scope("mid"):
            # layer-2 table: [dinv*h1 | zeros] fp16 rows, straight from ccout
            for run in runs:
                kind, rc0, n, l0 = run
                st = stg.tile([P, 12 * 128], f16, tag="stage")
                nc.vector.memset(st[:, :n * 128], 0.0)
                if kind != "zero":
                    pc = shp.tile([P, 12 * 64], f16, tag="h1pc")
                    nc.sync.dma_start(
                        pc[:, :n * 64],
                        ccout[kind, l0 * P:(l0 + n) * P, :]
                        .rearrange("(k p) f -> p k f", p=P))
                    for j in range(n):
                        nc.scalar.mul(st[:, j * 128:j * 128 + 64],
                                      pc[:, j * 64:(j + 1) * 64],
                                      dinvr_s[:, rc0 + j:rc0 + j + 1])
                tgt = g1tA if rc0 < c.ACH else g1tB
                r0 = rc0 - (0 if rc0 < c.ACH else c.ACH)
                nc.sync.dma_start(
                    tgt[r0 * P:(r0 + n) * P, :]
                    .rearrange("(k p) f -> p k f", p=P),
                    st[:, :n * 128])

            # root2 from local h1 (overlaps the collective / table build)
            root2 = big.tile([P, c.SHARD], f32, tag="root")
            for j in range(c.W):
                u2 = epi.tile([P, 64], f32, tag="u2")
                nc.scalar.copy(u2[:], h1n[:, j * 64:(j + 1) * 64])
                tp_ = psx.tile([P, 128], f32, tag="px")
                nc.tensor.transpose(out=tp_[:64, :], in_=u2[:],
                                    identity=ident[:])
                hl = epi.tile([64, 128], f32, tag="hl")
                nc.scalar.copy(hl[:], tp_[:64, :])
                ps = psy.tile([P, 128], f32, tag="pw2")
                nc.tensor.matmul(out=ps[:], lhsT=hl[:], rhs=rw2_s[:],
                                 start=True, stop=True)
                nc.vector.tensor_tensor(out=root2[:, j * 128:(j + 1) * 128],
                                        in0=ps[:], in1=b2_s[:], op=AL.add)

        yn = big.tile([P, c.W * 64], f32, tag="ht")
        with nc.named_scope("layer2"):
            layer(g1tA[:, :], g1tB[:, :], root2, yn, 128,
                  64, f16, w2_s)
        nc.sync.dma_start(yt[:, :].rearrange("(w p) f -> p w f", p=P), yn[:])

    nc.compile()
    return nc


_cache = {}


def prepare(inputs, cfg_kw=None):
    c = Cfg(**(cfg_kw or REAL))
    in_maps, NBA, NBB = _preprocess(c, **inputs)
    key = (tuple(sorted((cfg_kw or REAL).items())), tuple(NBA), tuple(NBB))
    if key not in _cache:
        _cache[key] = _build_program(c, NBA, NBB)
    return c, _cache[key], in_maps


def kernel(x, edge_index, init_w1, root_w1, b1, init_w2, root_w2, b2,
           _trace=False, _cfg=None):
    from concourse import bass_utils
    inputs = dict(x=np.asarray(x), edge_index=np.asarray(edge_index),
                  init_w1=np.asarray(init_w1), root_w1=np.asarray(root_w1),
                  b1=np.asarray(b1), init_w2=np.asarray(init_w2),
                  root_w2=np.asarray(root_w2), b2=np.asarray(b2))
    c, nc, in_maps = prepare(inputs, _cfg)
    res = bass_utils.run_bass_kernel_spmd(
        nc, in_maps, core_ids=list(range(c.NC)), trace=_trace)
    out = np.concatenate([res.results[cc]["yt"] for cc in range(c.NC)],
                         axis=0)[:c.N]
    if _trace:
        kernel._last = res
    return np.ascontiguousarray(out.astype(np.float32))
